# revision 22
# baseline (speedup 1.0000x reference)
"""APPNP (2-layer MLP + 2x K=10 PPR propagation) fully on 8 TRN2 cores.

Design (v2 — full device, replaces the host-propagation baseline):
- Nodes padded to 100352 = 8*12544; core c owns dst rows [c*12544,(c+1)*12544).
- Scaled state m = dinv*x lives in SBUF (fp32) per core; the full node
  table (all-gathered every hop, fp32 [100352, 64], 256B rows) ping-pongs
  between two Shared DRAM buffers.
- Per hop per core: messages m[src] are fetched with GPSIMD swdge
  dma_gather (1024 int16 idxs/call, source chunked 4x25088 to fit int16)
  and accumulated into 4 per-chunk HBM accumulators with dma_scatter_add
  (idx = dst_local, pads land in trash rows >= 12544). Accumulators are
  pre-initialized with t/(4*c1) so the teleport term folds in for free:
  m' = c1*(sum_q acc_q + m). AllGather of the updated shard feeds the
  next hop.
- GEMMs, relu, and log_softmax all run on device; the kernel launches a
  single compiled program once (no per-hop host round trips).
- Numpy fallback keeps correctness if the device path fails.
"""
import os
import sys

import numpy as np

sys.path.insert(0, '/opt/trn_rl_repo')

N = 100000
E = 1600000
F_IN = 128
F_HID = 64
F_OUT = 40
K_HOPS = 10
ALPHA = 0.1

NC = 8
SHARD = 12544            # 98 * 128
NPAD = NC * SHARD        # 100352
NB = SHARD // 128        # 98 row-blocks per core
CHUNK = 25088            # NPAD / 4, < int16 max
NQ = 4
ACC_ROWS = SHARD + 256   # trash rows for padded edges
SLICE_I = 1024           # idxs per swdge call (hard ucode limit)
GB = 7                   # row-blocks per update group (98 = 14*7)

_cache = {}


# ----------------------------------------------------------------- host prep
def _preprocess(edge_index):
    src = np.asarray(edge_index[0], dtype=np.int64)
    dst = np.asarray(edge_index[1], dtype=np.int64)
    deg = np.bincount(dst, minlength=NPAD).astype(np.float32) + 1.0
    real = np.zeros(NPAD, dtype=bool)
    real[:N] = True
    dinv = np.where(real, 1.0 / np.sqrt(deg), 0.0).astype(np.float32)
    rdinv = np.where(real, np.sqrt(deg), 0.0).astype(np.float32)

    owner = dst // SHARD
    q = src // CHUNK
    key = owner * NQ + q
    # owner-major, chunk, then dst. Within each (core, chunk) stream edges
    # are dst-sorted; consecutive same-dst edges form PAIRS whose messages
    # are pre-summed on DVE before the scatter (halves scatter idx work);
    # odd leftovers go to a "singles" region. Pairs are round-robin dealt
    # across scatter calls so every call has distinct dst rows
    # (dma_scatter_add loses concurrent same-row adds within a call).
    order = np.lexsort((dst, q, owner))
    ssrc = src[order]
    sdst = dst[order]
    counts = np.bincount(key, minlength=NC * NQ)
    bounds = np.zeros(NC * NQ + 1, dtype=np.int64)
    np.cumsum(counts, out=bounds[1:])

    # first pass: pair/single split per stream to size GP/GS uniformly
    splits = {}
    np_max = ns_max = 0
    for c in range(NC):
        for qq in range(NQ):
            k = c * NQ + qq
            sl = (sdst[bounds[k]:bounds[k + 1]] % SHARD).astype(np.int64)
            n = len(sl)
            st = np.flatnonzero(np.r_[True, sl[1:] != sl[:-1]])
            rl = np.diff(np.r_[st, n])
            rid = np.repeat(np.arange(len(st)), rl)
            pos = np.arange(n) - st[rid]
            rlv = rl[rid]
            single = (pos == rlv - 1) & (rlv % 2 == 1)
            i0 = np.flatnonzero(~single & (pos % 2 == 0))
            sg = np.flatnonzero(single)
            splits[k] = (i0, sg)
            np_max = max(np_max, len(i0))
            ns_max = max(ns_max, len(sg))
    SP = int(np.ceil(np_max / SLICE_I))      # paired scatter calls
    GP = 2 * SP                              # paired gather calls
    GS = int(np.ceil(ns_max / SLICE_I))      # singles calls (gather+scatter)

    def wrap(a):
        return np.tile(a.reshape(-1, 16).T, (8, 1)).astype(np.int16)

    per_core = []
    trashp = (12544 + (np.arange(SP * SLICE_I) % 256)).astype(np.int16)
    trashs = (12544 + (np.arange(GS * SLICE_I) % 256)).astype(np.int16)
    for c in range(NC):
        gws, spws, ssws = [], [], []
        for qq in range(NQ):
            k = c * NQ + qq
            gl = (ssrc[bounds[k]:bounds[k + 1]] % CHUNK).astype(np.int16)
            sl = (sdst[bounds[k]:bounds[k + 1]] % SHARD).astype(np.int16)
            i0, sg = splits[k]
            npair, ns = len(i0), len(sg)
            # same-dst pair-run must fit in SP distinct calls
            if npair:
                pr = sl[i0]
                runs = np.diff(np.flatnonzero(np.r_[True, pr[1:] != pr[:-1],
                                                    True]))
                assert runs.max() <= SP, (runs.max(), SP)
            t = np.arange(npair)
            cpos = t % SP
            u = t // SP
            g = 2 * cpos + u // 512
            v = u % 512
            gflat = np.zeros((GP + GS) * SLICE_I, dtype=np.int16)
            gflat[g * SLICE_I + v] = gl[i0]
            gflat[g * SLICE_I + 512 + v] = gl[i0 + 1]
            spflat = trashp.copy()
            spflat[cpos * SLICE_I + u] = sl[i0]
            gflat[GP * SLICE_I + np.arange(ns)] = gl[sg]
            ssflat = trashs.copy()
            ssflat[:ns] = sl[sg]
            gws.append(wrap(gflat))
            spws.append(wrap(spflat))
            ssws.append(wrap(ssflat))
        lo = c * SHARD
        dv = dinv[lo:lo + SHARD].reshape(NB, 128).T.copy()
        rv = rdinv[lo:lo + SHARD].reshape(NB, 128).T.copy()
        c1 = (0.9 * dv * dv).astype(np.float32)
        tpc = (0.25 * (ALPHA / 0.9) * rv).astype(np.float32)
        per_core.append({
            "gidx": np.concatenate(gws, axis=1),
            "sidxp": np.concatenate(spws, axis=1),
            "sidxs": np.concatenate(ssws, axis=1),
            "dinv": np.ascontiguousarray(dv),
            "rdinv": np.ascontiguousarray(rv),
            "c1": np.ascontiguousarray(c1),
            "tpc": np.ascontiguousarray(tpc),
        })
    return per_core, (GP, GS)


# -------------------------------------------------------------- bass program
def _build(dims):
    from concourse import bass, bacc, tile, mybir

    f32 = mybir.dt.float32
    i16 = mybir.dt.int16
    i32 = mybir.dt.int32
    Alu = mybir.AluOpType
    Act = mybir.ActivationFunctionType

    GP, GS = dims
    SP = GP // 2
    CW = SLICE_I // 16   # idx cols per call

    nc = bacc.Bacc("TRN2", target_bir_lowering=False, debug=False,
                   enable_asserts=False, num_devices=NC)

    xT = nc.dram_tensor("xT", [F_IN, SHARD], f32, kind="ExternalInput").ap()
    w1 = nc.dram_tensor("w1", [F_IN, F_HID], f32, kind="ExternalInput").ap()
    b1 = nc.dram_tensor("b1", [128, F_HID], f32, kind="ExternalInput").ap()
    w2 = nc.dram_tensor("w2", [F_HID, F_HID], f32, kind="ExternalInput").ap()
    b2 = nc.dram_tensor("b2", [128, F_HID], f32, kind="ExternalInput").ap()
    gidx = nc.dram_tensor("gidx", [128, NQ * (GP + GS) * CW], i16,
                          kind="ExternalInput").ap()
    sidxp = nc.dram_tensor("sidxp", [128, NQ * SP * CW], i16,
                           kind="ExternalInput").ap()
    sidxs = nc.dram_tensor("sidxs", [128, max(NQ * GS * CW, 16)], i16,
                           kind="ExternalInput").ap()
    dinv = nc.dram_tensor("dinv", [128, NB], f32, kind="ExternalInput").ap()
    rdinv = nc.dram_tensor("rdinv", [128, NB], f32, kind="ExternalInput").ap()
    c1 = nc.dram_tensor("c1", [128, NB], f32, kind="ExternalInput").ap()
    tpc = nc.dram_tensor("tpc", [128, NB], f32, kind="ExternalInput").ap()
    y = nc.dram_tensor("y", [SHARD, F_OUT], f32, kind="ExternalOutput").ap()

    tabA = nc.dram_tensor("tabA", [NPAD, F_HID], f32, kind="Internal",
                          addr_space="Shared").ap()
    tabB = nc.dram_tensor("tabB", [NPAD, F_HID], f32, kind="Internal",
                          addr_space="Shared").ap()
    bounce = nc.dram_tensor("bounce", [SHARD, F_HID], f32,
                            kind="Internal").ap()
    accs = [nc.dram_tensor(f"acc{a}", [ACC_ROWS, F_HID], f32,
                           kind="Internal").ap() for a in range(NQ)]
    tabs = [tabA, tabB]

    with tile.TileContext(nc) as tc:
        with tc.tile_pool(name="fix", bufs=1) as fix, \
             tc.tile_pool(name="state", bufs=1) as stp, \
             tc.tile_pool(name="sbuf", bufs=15) as pool, \
             tc.tile_pool(name="upd", bufs=2) as updp, \
             tc.tile_pool(name="psum", bufs=4, space="PSUM") as psum:

            # ---- fixed tiles
            gidx_t = fix.tile([128, NQ * (GP + GS) * CW], i16)
            nc.sync.dma_start(out=gidx_t[:], in_=gidx[:])
            sidxp_t = fix.tile([128, NQ * SP * CW], i16)
            nc.sync.dma_start(out=sidxp_t[:], in_=sidxp[:])
            sidxs_t = fix.tile([128, max(NQ * GS * CW, 16)], i16)
            nc.sync.dma_start(out=sidxs_t[:], in_=sidxs[:, 0:max(NQ * GS * CW,
                                                                 16)])
            dinv_t = fix.tile([128, NB], f32)
            nc.sync.dma_start(out=dinv_t[:], in_=dinv[:])
            rdinv_t = fix.tile([128, NB], f32)
            nc.sync.dma_start(out=rdinv_t[:], in_=rdinv[:])
            c1_t = fix.tile([128, NB], f32)
            nc.sync.dma_start(out=c1_t[:], in_=c1[:])
            tpc_t = fix.tile([128, NB], f32)
            nc.sync.dma_start(out=tpc_t[:], in_=tpc[:])
            w1_t = fix.tile([F_IN, F_HID], f32)
            nc.sync.dma_start(out=w1_t[:], in_=w1[:])
            b1_t = fix.tile([128, F_HID], f32)
            nc.sync.dma_start(out=b1_t[:], in_=b1[:])
            w2_t = fix.tile([F_HID, F_HID], f32)
            nc.sync.dma_start(out=w2_t[:], in_=w2[:])
            b2_t = fix.tile([128, F_HID], f32)
            nc.sync.dma_start(out=b2_t[:], in_=b2[:])

            # identity for PE transpose
            iota_r = fix.tile([128, 128], i32)
            nc.gpsimd.iota(iota_r[:], pattern=[[1, 128]], base=0,
                           channel_multiplier=0)
            iota_rf = fix.tile([128, 128], f32)
            nc.vector.tensor_scalar_add(iota_rf[:], iota_r[:], 0.0)
            iota_c = fix.tile([128, 1], i32)
            nc.gpsimd.iota(iota_c[:], pattern=[[0, 1]], base=0,
                           channel_multiplier=1)
            iota_cf = fix.tile([128, 1], f32)
            nc.vector.tensor_scalar_add(iota_cf[:], iota_c[:], 0.0)
            ident_t = fix.tile([128, 128], f32)
            nc.vector.tensor_scalar(out=ident_t[:], in0=iota_rf[:],
                                    scalar1=iota_cf[:], scalar2=None,
                                    op0=Alu.is_equal)

            # persistent state
            m_t = stp.tile([128, NB, F_HID], f32)     # m = dinv * x
            tq_t = stp.tile([128, NB, F_HID], f32)    # t_pre / 4

            def allgather(tab_dst):
                nc.sync.dma_start(
                    out=bounce.rearrange("(b p) f -> p b f", p=128),
                    in_=m_t[:])
                nc.gpsimd.collective_compute(
                    "AllGather", Alu.bypass,
                    replica_groups=[list(range(NC))],
                    ins=[bounce.opt()], outs=[tab_dst.opt()])

            # ---- GEMM1: h1 = relu(x @ W1 + b1); m0 = dinv*h1; tq = tpc*h1
            for b in range(NB):
                lx = pool.tile([F_IN, 128], f32, tag="lx")
                nc.sync.dma_start(out=lx[:], in_=xT[:, b * 128:(b + 1) * 128])
                pt = psum.tile([128, F_HID], f32, tag="pg")
                nc.tensor.matmul(out=pt[:], lhsT=lx[:], rhs=w1_t[:],
                                 start=True, stop=True)
                h = pool.tile([128, F_HID], f32, tag="h")
                nc.vector.tensor_tensor(out=h[:], in0=pt[:], in1=b1_t[:],
                                        op=Alu.add)
                nc.vector.tensor_scalar_max(h[:], h[:], 0.0)
                nc.vector.tensor_scalar_mul(m_t[:, b, :], h[:],
                                            dinv_t[:, b:b + 1])
                nc.vector.tensor_scalar_mul(tq_t[:, b, :], h[:],
                                            tpc_t[:, b:b + 1])
            allgather(tabA)

            # ---- one propagation hop
            def hop(tsrc, tdst, do_ag=True):
                for a in range(NQ):
                    nc.sync.dma_start(
                        out=accs[a][0:SHARD, :].rearrange(
                            "(b p) f -> p b f", p=128),
                        in_=tq_t[:])
                # paired region: two gathers -> DVE pair-sum -> one scatter
                for cc in range(SP):
                    for a in range(NQ):
                        qb = a * (GP + GS)
                        g0 = pool.tile([128, 8, F_HID], f32, tag="msg")
                        nc.gpsimd.dma_gather(
                            g0[:], tsrc[a * CHUNK:(a + 1) * CHUNK, :],
                            gidx_t[:, (qb + 2 * cc) * CW:
                                   (qb + 2 * cc + 1) * CW],
                            SLICE_I, SLICE_I, F_HID)
                        g1 = pool.tile([128, 8, F_HID], f32, tag="msg")
                        nc.gpsimd.dma_gather(
                            g1[:], tsrc[a * CHUNK:(a + 1) * CHUNK, :],
                            gidx_t[:, (qb + 2 * cc + 1) * CW:
                                   (qb + 2 * cc + 2) * CW],
                            SLICE_I, SLICE_I, F_HID)
                        cp = pool.tile([128, 8, F_HID], f32, tag="cmp")
                        nc.vector.tensor_tensor(
                            out=cp[:, 0:4, :], in0=g0[:, 0:4, :],
                            in1=g0[:, 4:8, :], op=Alu.add)
                        nc.vector.tensor_tensor(
                            out=cp[:, 4:8, :], in0=g1[:, 0:4, :],
                            in1=g1[:, 4:8, :], op=Alu.add)
                        nc.gpsimd.dma_scatter_add(
                            accs[a], cp[:],
                            sidxp_t[:, (a * SP + cc) * CW:
                                    (a * SP + cc + 1) * CW],
                            SLICE_I, SLICE_I, F_HID)
                # singles region: direct gather -> scatter
                for s in range(GS):
                    for a in range(NQ):
                        qb = a * (GP + GS)
                        g = pool.tile([128, 8, F_HID], f32, tag="msg")
                        nc.gpsimd.dma_gather(
                            g[:], tsrc[a * CHUNK:(a + 1) * CHUNK, :],
                            gidx_t[:, (qb + GP + s) * CW:
                                   (qb + GP + s + 1) * CW],
                            SLICE_I, SLICE_I, F_HID)
                        nc.gpsimd.dma_scatter_add(
                            accs[a], g[:],
                            sidxs_t[:, (a * GS + s) * CW:
                                    (a * GS + s + 1) * CW],
                            SLICE_I, SLICE_I, F_HID)
                for gi in range(NB // GB):
                    rows = slice(gi * GB * 128, (gi + 1) * GB * 128)
                    ats = []
                    for a in range(NQ):
                        at = updp.tile([128, GB, F_HID], f32, tag=f"a{a}")
                        nc.sync.dma_start(
                            out=at[:],
                            in_=accs[a][rows, :].rearrange(
                                "(b p) f -> p b f", p=128))
                        ats.append(at)
                    s1 = updp.tile([128, GB, F_HID], f32, tag="s1")
                    nc.vector.tensor_tensor(out=s1[:], in0=ats[0][:],
                                            in1=ats[1][:], op=Alu.add)
                    s2 = updp.tile([128, GB, F_HID], f32, tag="s2")
                    nc.vector.tensor_tensor(out=s2[:], in0=ats[2][:],
                                            in1=ats[3][:], op=Alu.add)
                    nc.vector.tensor_tensor(out=s1[:], in0=s1[:], in1=s2[:],
                                            op=Alu.add)
                    nc.vector.tensor_tensor(
                        out=s1[:], in0=s1[:],
                        in1=m_t[:, gi * GB:(gi + 1) * GB, :], op=Alu.add)
                    for j in range(GB):
                        b = gi * GB + j
                        nc.vector.tensor_scalar_mul(
                            m_t[:, b, :], s1[:, j, :], c1_t[:, b:b + 1])
                if do_ag:
                    allgather(tdst)

            for h_i in range(K_HOPS):
                hop(tabs[h_i % 2], tabs[(h_i + 1) % 2],
                    do_ag=(h_i < K_HOPS - 1))

            # ---- GEMM2: x10 = m*rdinv; h2 = relu(x10 @ W2 + b2)
            for b in range(NB):
                xb = pool.tile([128, F_HID], f32, tag="xb")
                nc.vector.tensor_scalar_mul(xb[:], m_t[:, b, :],
                                            rdinv_t[:, b:b + 1])
                ptr = psum.tile([F_HID, 128], f32, tag="ptr")
                nc.tensor.transpose(ptr[:], xb[:], ident_t[:])
                lT = pool.tile([F_HID, 128], f32, tag="lT")
                nc.scalar.copy(out=lT[:], in_=ptr[:])
                p2 = psum.tile([128, F_HID], f32, tag="pg")
                nc.tensor.matmul(out=p2[:], lhsT=lT[:], rhs=w2_t[:],
                                 start=True, stop=True)
                h = pool.tile([128, F_HID], f32, tag="h")
                nc.vector.tensor_tensor(out=h[:], in0=p2[:], in1=b2_t[:],
                                        op=Alu.add)
                nc.vector.tensor_scalar_max(h[:], h[:], 0.0)
                nc.vector.tensor_scalar_mul(m_t[:, b, :], h[:],
                                            dinv_t[:, b:b + 1])
                nc.vector.tensor_scalar_mul(tq_t[:, b, :], h[:],
                                            tpc_t[:, b:b + 1])
            allgather(tabA)

            for h_i in range(K_HOPS):
                hop(tabs[h_i % 2], tabs[(h_i + 1) % 2],
                    do_ag=(h_i < K_HOPS - 1))

            # ---- log_softmax over first F_OUT cols
            for b in range(NB):
                xf = pool.tile([128, F_OUT], f32, tag="xf")
                nc.vector.tensor_scalar_mul(xf[:], m_t[:, b, 0:F_OUT],
                                            rdinv_t[:, b:b + 1])
                mx = pool.tile([128, 1], f32, tag="mx")
                nc.vector.tensor_reduce(out=mx[:], in_=xf[:],
                                        axis=mybir.AxisListType.X, op=Alu.max)
                mxn = pool.tile([128, 1], f32, tag="mxn")
                nc.vector.tensor_scalar_mul(mxn[:], mx[:], -1.0)
                ex = pool.tile([128, F_OUT], f32, tag="ex")
                nc.scalar.activation(out=ex[:], in_=xf[:], func=Act.Exp,
                                     bias=mxn[:])
                sm = pool.tile([128, 1], f32, tag="sm")
                nc.vector.tensor_reduce(out=sm[:], in_=ex[:],
                                        axis=mybir.AxisListType.X, op=Alu.add)
                ls = pool.tile([128, 1], f32, tag="ls")
                nc.scalar.activation(out=ls[:], in_=sm[:], func=Act.Ln)
                fin = pool.tile([128, F_OUT], f32, tag="fin")
                nc.vector.tensor_scalar(out=fin[:], in0=xf[:],
                                        scalar1=mx[:], scalar2=ls[:],
                                        op0=Alu.subtract, op1=Alu.subtract)
                nc.sync.dma_start(out=y[b * 128:(b + 1) * 128, :], in_=fin[:])
    nc.compile()
    return nc


# ------------------------------------------------------------------- driver
def _run_device(inputs, trace=False):
    from concourse import bass_utils

    x = np.asarray(inputs["x"], dtype=np.float32)
    W1 = np.asarray(inputs["W1"], dtype=np.float32)
    b1 = np.asarray(inputs["b1"], dtype=np.float32)
    W2 = np.asarray(inputs["W2"], dtype=np.float32)
    b2 = np.asarray(inputs["b2"], dtype=np.float32)
    edge_index = np.asarray(inputs["edge_index"])

    per_core, dims = _preprocess(edge_index)
    key = ("prog", dims)
    if key not in _cache:
        _cache[key] = _build(dims)
    nc = _cache[key]

    xp = np.zeros((NPAD, F_IN), dtype=np.float32)
    xp[:N] = x
    w1p = W1
    b1p = np.tile(b1[None, :], (128, 1)).astype(np.float32)
    w2p = np.zeros((F_HID, F_HID), dtype=np.float32)
    w2p[:, :F_OUT] = W2
    b2p = np.zeros((128, F_HID), dtype=np.float32)
    b2p[:, :F_OUT] = b2

    in_maps = []
    for c in range(NC):
        pc = per_core[c]
        in_maps.append({
            "xT": np.ascontiguousarray(xp[c * SHARD:(c + 1) * SHARD].T),
            "w1": w1p, "b1": b1p, "w2": w2p, "b2": b2p,
            "gidx": pc["gidx"], "sidxp": pc["sidxp"], "sidxs": pc["sidxs"],
            "dinv": pc["dinv"], "rdinv": pc["rdinv"],
            "c1": pc["c1"], "tpc": pc["tpc"],
        })
    res = bass_utils.run_bass_kernel_spmd(
        nc, in_maps, core_ids=list(range(NC)), trace=trace)
    out = np.concatenate([res.results[c]["y"] for c in range(NC)], axis=0)
    return out[:N], res


# ------------------------------------------------------------ numpy fallback
def _numpy_ref(x, edge_index, W1, b1, W2, b2):
    src = edge_index[0].astype(np.int64)
    dst = edge_index[1].astype(np.int64)
    deg = np.bincount(dst, minlength=N).astype(np.float32) + 1.0
    dinv = 1.0 / np.sqrt(deg)

    def prop(h):
        m = dinv[:, None] * h
        c1 = 0.9 * dinv * dinv
        t = ALPHA * dinv[:, None] * h
        for _ in range(K_HOPS):
            agg = np.zeros_like(m)
            np.add.at(agg, dst, m[src])
            m = c1[:, None] * (agg + m) + t
        return m / dinv[:, None]

    h = np.maximum(x @ W1 + b1, 0.0)
    h = prop(h)
    h = np.maximum(h @ W2 + b2, 0.0)
    h = prop(h)
    mx = h.max(axis=1, keepdims=True)
    e = np.exp(h - mx)
    return (h - mx) - np.log(e.sum(axis=1, keepdims=True))


def kernel(x, edge_index, W1, b1, W2, b2):
    inputs = {"x": x, "edge_index": edge_index, "W1": W1, "b1": b1,
              "W2": W2, "b2": b2}
    try:
        out, _ = _run_device(inputs, trace=False)
        return out.astype(np.float32)
    except Exception as exc:  # device path failed -> numpy fallback
        print(f"kernel: device path failed ({exc!r}); numpy fallback",
              file=sys.stderr)
        return _numpy_ref(np.asarray(x, np.float32), np.asarray(edge_index),
                          np.asarray(W1, np.float32),
                          np.asarray(b1, np.float32),
                          np.asarray(W2, np.float32),
                          np.asarray(b2, np.float32)).astype(np.float32)
